# revision 8
# baseline (speedup 1.0000x reference)
"""Trainium2 Bass kernel: 3x3 valid conv (64ch -> 128ch) + per-pixel bias.

Strategy: shard the 510 output rows spatially across 8 NeuronCores (64
rows/core with a 2-row input halo; core 7 overlaps core 6 by 2 rows).
Inside a core, the 64-row band is split across the two PE row-strips:
partitions 0-63 hold the input rows for output rows 0-31 of the band,
partitions 64-127 the rows for output rows 32-63 (the host feeds the
band pre-split so every DMA runs at full 128-partition width).  Each
output row is 9 accumulating K=64 matmuls (one per kernel tap, N=510);
the two strips run concurrently on disjoint PE row-halves, so a
tap-pair costs one N=510 stream (~213ns warm) and the 288 pair-streams
put the PE at its 61us roofline for this shard.

Everything rides HBM as fp16: the PE streams 16-bit operands at the
same 1 col/cycle as fp32r, so halving every tensor's bytes moves the
kernel from DMA-bound (34.6MB at the ~420 GB/s fabric cap = 82us) to
PE-bound (21.5MB = 51us of DMA hidden under the 62us matmul stream).
fp16's 10 mantissa bits keep the end-to-end error ~1e-3 of output
absmax.  fp16 weights also enable the compiler's fast-weight-load path
(32-bit reads), so the per-tap LDWEIGHTS fully hides under the stream.

Ring plan: sync carries w + all bias + strip-a stores; scalar carries
the input chunks (first-matmul critical path) + strip-b stores.  Bias
is fully buffered in SBUF (8 group buffers per strip) so every PSUM
evacuation is a single fused DVE add; the last two groups store row by
row on both rings so the final drain starts as soon as each row lands.
PSUM is accumulated in fp32 and only rounded to fp16 once, at the
bias-add.  Output is converted back to fp32 on the host.
"""

import numpy as np
from contextlib import ExitStack

import concourse.bass as bass
import concourse.tile as tile
from concourse import bacc, mybir
from concourse import bass_utils

C, H, W = 64, 512, 512
D, KK = 128, 3
OH, OW = H - KK + 1, W - KK + 1          # 510, 510
NCORES = 8
RPC = 64                                  # output rows per core
BAND = RPC + KK - 1                       # 66 input rows per core
HALF = RPC // 2                           # 32 output rows per strip
IBAND = HALF + KK - 1                     # 34 input rows per strip
GROUPS = 8
GROWS = HALF // GROUPS                    # 4 pair-rows per group

f32 = mybir.dt.float32
f16 = mybir.dt.float16

# row offset of each core's output band
STARTS = [min(i * RPC, OH - RPC) for i in range(NCORES)]

_CACHE = {}

# results of the last hardware run (inspected by test harnesses)
LAST_RESULTS = None


def _build_program():
    nc = bacc.Bacc(
        "TRN2", target_bir_lowering=False, debug=False, num_devices=NCORES
    )
    # x is pre-split on the host: row (h*64+c) holds band rows
    # [32h, 32h+34) of channel c, flattened
    x = nc.dram_tensor("x", [2 * C, IBAND * W], f16, kind="ExternalInput").ap()
    # w is pre-duplicated: rows 0-63 and 64-127 identical, [c, (ky kx d)]
    w = nc.dram_tensor("w", [2 * C, 9 * D], f16, kind="ExternalInput").ap()
    b = nc.dram_tensor("b", [D, RPC, OW], f16, kind="ExternalInput").ap()
    y = nc.dram_tensor("y", [D, RPC, OW], f16, kind="ExternalOutput").ap()

    b_flat = b.rearrange("d r x -> d (r x)")
    y_flat = y.rearrange("d r x -> d (r x)")

    with tile.TileContext(nc) as tc:
        with ExitStack() as ctx:
            xp = ctx.enter_context(tc.tile_pool(name="xin", bufs=1))
            wp = ctx.enter_context(tc.tile_pool(name="wt", bufs=1))
            bp = ctx.enter_context(tc.tile_pool(name="bias", bufs=8))
            op = ctx.enter_context(tc.tile_pool(name="out", bufs=4))
            pp = ctx.enter_context(tc.tile_pool(name="ps", bufs=4, space="PSUM"))

            # HAM pre-warm: the PE clock sits at 1.2 GHz until ~3.4us of
            # sustained matmul activity.  Burn that window on dummy
            # matmuls over a memset scratch tile while the input DMA is
            # in flight, so the real stream runs at 2.4 GHz from pair 0.
            # The dummies alternate between the two PE row-halves so each
            # LDWEIGHTS pulls ahead of the in-flight matmul (~110ns per
            # dummy); the PSUM target shares the pa pool's first buffer.
            sc = wp.tile([128, 128], f16, tag="scratch")
            nc.gpsimd.memset(sc[:], 0.0)
            wps_a = pp.tile([128, OW], f32, tag="pa")
            wps_b = pp.tile([128, OW], f32, tag="pb")
            for i in range(34):
                h = slice(0, 64) if i % 2 == 0 else slice(64, 128)
                wps = wps_a if i % 2 == 0 else wps_b
                nc.tensor.matmul(
                    wps[:, 0:128], sc[h, :], sc[h, 0:128],
                    start=True, stop=True,
                )

            # critical path to the first matmul: weights on sync, first
            # input chunk on scalar, concurrently
            wt = wp.tile([128, 9 * D], f16)
            nc.sync.dma_start(wt[:], w[:, :])

            xin = xp.tile([128, IBAND * W], f16)
            bounds = [0, 2, 5, 10, 16, 24, IBAND]
            for ci in range(len(bounds) - 1):
                r0, r1 = bounds[ci], bounds[ci + 1]
                nc.scalar.dma_start(
                    xin[:, r0 * W:r1 * W], x[:, r0 * W:r1 * W]
                )

            # bias rides both rings, interleaved with the stores so each
            # ring's FIFO byte order matches production order: groups 0-1
            # fully up front on sync (scalar is busy with x), then group
            # g+2's pair issues just before group g's matmuls
            bias_tiles = {}

            def load_bias(g, eng_a, eng_b):
                ra, rb = g * GROWS, HALF + g * GROWS
                ba = bp.tile([128, GROWS * OW], f16, tag="ba")
                eng_a.dma_start(ba[:], b_flat[:, ra * OW:(ra + GROWS) * OW])
                bb = bp.tile([128, GROWS * OW], f16, tag="bb")
                eng_b.dma_start(bb[:], b_flat[:, rb * OW:(rb + GROWS) * OW])
                bias_tiles[g] = (ba, bb)

            load_bias(0, nc.sync, nc.sync)
            load_bias(1, nc.sync, nc.sync)

            for g in range(GROUPS):
                ra = g * GROWS                 # band rows ra..ra+3  (strip 0)
                rb = HALF + ra                 # band rows rb..rb+3  (strip 1)
                if g + 2 < GROUPS:
                    load_bias(g + 2, nc.sync, nc.scalar)
                ba, bb = bias_tiles.pop(g)
                ya = op.tile([128, GROWS * OW], f16, tag="ya")
                yb = op.tile([128, GROWS * OW], f16, tag="yb")

                tail = g >= GROUPS - 2
                for j in range(GROWS):
                    yl = ra + j                # strip-local output row
                    pa = pp.tile([128, OW], f32, tag="pa")
                    pb = pp.tile([128, OW], f32, tag="pb")
                    for t in range(9):
                        ky, kx = divmod(t, 3)
                        off = (yl + ky) * W + kx
                        nc.tensor.matmul(
                            pa[:],
                            wt[0:64, t * D:(t + 1) * D],
                            xin[0:64, off:off + OW],
                            start=(t == 0), stop=(t == 8),
                        )
                        nc.tensor.matmul(
                            pb[:],
                            wt[64:128, t * D:(t + 1) * D],
                            xin[64:128, off:off + OW],
                            start=(t == 0), stop=(t == 8),
                        )
                    sl = slice(j * OW, (j + 1) * OW)
                    if g == GROUPS - 1 and j == GROWS - 1:
                        # the very last row: column-split the bias-add and
                        # the store so the final HBM write starts one DVE
                        # half-add (not two full adds) after the last
                        # matmul, interleaved across both rings
                        HC = OW // 2
                        for c0, c1 in ((0, HC), (HC, OW)):
                            ch = slice(j * OW + c0, j * OW + c1)
                            nc.vector.tensor_add(
                                yb[:, ch], pb[:, c0:c1], bb[:, ch]
                            )
                            nc.sync.dma_start(
                                y_flat[:, (rb + j) * OW + c0:
                                       (rb + j) * OW + c1],
                                yb[:, ch],
                            )
                            nc.vector.tensor_add(
                                ya[:, ch], pa[:, c0:c1], ba[:, ch]
                            )
                            nc.scalar.dma_start(
                                y_flat[:, (ra + j) * OW + c0:
                                       (ra + j) * OW + c1],
                                ya[:, ch],
                            )
                    else:
                        nc.vector.tensor_add(ya[:, sl], pa[:], ba[:, sl])
                        nc.vector.tensor_add(yb[:, sl], pb[:], bb[:, sl])
                        if tail:
                            # last two groups: store each row as soon as
                            # its add lands, one strip per ring
                            nc.scalar.dma_start(
                                y_flat[:, (ra + j) * OW:(ra + j + 1) * OW],
                                ya[:, sl],
                            )
                            nc.sync.dma_start(
                                y_flat[:, (rb + j) * OW:(rb + j + 1) * OW],
                                yb[:, sl],
                            )

                if not tail:
                    nc.sync.dma_start(
                        y_flat[:, ra * OW:(ra + GROWS) * OW], ya[:]
                    )
                    nc.scalar.dma_start(
                        y_flat[:, rb * OW:(rb + GROWS) * OW], yb[:]
                    )

    nc.compile()
    return nc


def kernel(input, kernels, biases):
    global LAST_RESULTS
    if "nc" not in _CACHE:
        _CACHE["nc"] = _build_program()
    nc = _CACHE["nc"]

    xr = np.asarray(input, dtype=np.float16)                   # [C, H, W]
    w1 = np.ascontiguousarray(
        np.asarray(kernels, dtype=np.float32).transpose(1, 2, 3, 0)
    ).reshape(C, 9 * D).astype(np.float16)
    wr = np.concatenate([w1, w1], axis=0)                      # [128, 9*D]
    br = np.asarray(biases, dtype=np.float16)

    in_maps = []
    for s in STARTS:
        band = xr[:, s:s + BAND, :]
        xs = np.concatenate(
            [band[:, 0:IBAND, :], band[:, HALF:HALF + IBAND, :]], axis=0
        ).reshape(2 * C, IBAND * W)
        in_maps.append({
            "x": np.ascontiguousarray(xs),
            "w": wr,
            "b": np.ascontiguousarray(br[:, s:s + RPC, :]),
        })

    res = bass_utils.run_bass_kernel_spmd(
        nc, in_maps, core_ids=list(range(NCORES))
    )
    LAST_RESULTS = res

    out = np.empty((D, OH, OW), np.float32)
    for i, s in enumerate(STARTS):
        out[:, s:s + RPC, :] = res.results[i]["y"].astype(np.float32)
    return out


# revision 9
# speedup vs baseline: 1.0457x; 1.0457x over previous
"""Trainium2 Bass kernel: 3x3 valid conv (64ch -> 128ch) + per-pixel bias.

Strategy: shard the 510 output rows spatially across 8 NeuronCores (64
rows/core with a 2-row input halo; core 7 overlaps core 6 by 2 rows).
Inside a core, the 64-row band is split across the two PE row-strips:
partitions 0-63 hold the input rows for output rows 0-31 of the band,
partitions 64-127 the rows for output rows 32-63 (the host feeds the
band pre-split so every DMA runs at full 128-partition width).  Each
output row is 9 accumulating K=64 matmuls (one per kernel tap, N=510);
the two strips run concurrently on disjoint PE row-halves, so a
tap-pair costs one N=510 stream (~213ns warm) and the 288 pair-streams
put the PE at its 61us roofline for this shard.

Everything rides HBM as fp16: the PE streams 16-bit operands at the
same 1 col/cycle as fp32r, so halving every tensor's bytes moves the
kernel from DMA-bound (34.6MB at the ~420 GB/s fabric cap = 82us) to
PE-bound (21.5MB = 51us of DMA hidden under the 62us matmul stream).
fp16's 10 mantissa bits keep the end-to-end error ~1e-3 of output
absmax.  fp16 weights also enable the compiler's fast-weight-load path
(32-bit reads), so the per-tap LDWEIGHTS fully hides under the stream.

Ring plan: sync carries w + all bias + strip-a stores; scalar carries
the input chunks (first-matmul critical path) + strip-b stores.  Bias
is fully buffered in SBUF (8 group buffers per strip) so every PSUM
evacuation is a single fused DVE add; the last two groups store row by
row on both rings so the final drain starts as soon as each row lands.
PSUM is accumulated in fp32 and only rounded to fp16 once, at the
bias-add.  Output is converted back to fp32 on the host.
"""

import numpy as np
from contextlib import ExitStack

import concourse.bass as bass
import concourse.tile as tile
from concourse import bacc, mybir
from concourse import bass_utils

C, H, W = 64, 512, 512
D, KK = 128, 3
OH, OW = H - KK + 1, W - KK + 1          # 510, 510
NCORES = 8
RPC = 64                                  # output rows per core
BAND = RPC + KK - 1                       # 66 input rows per core
HALF = RPC // 2                           # 32 output rows per strip
IBAND = HALF + KK - 1                     # 34 input rows per strip
GROUPS = 8
GROWS = HALF // GROUPS                    # 4 pair-rows per group

f32 = mybir.dt.float32
f16 = mybir.dt.float16

# row offset of each core's output band
STARTS = [min(i * RPC, OH - RPC) for i in range(NCORES)]

_CACHE = {}

# results of the last hardware run (inspected by test harnesses)
LAST_RESULTS = None


def _build_program():
    nc = bacc.Bacc(
        "TRN2", target_bir_lowering=False, debug=False, num_devices=NCORES
    )
    # x is pre-split on the host: row (h*64+c) holds band rows
    # [32h, 32h+34) of channel c, flattened
    x = nc.dram_tensor("x", [2 * C, IBAND * W], f16, kind="ExternalInput").ap()
    # w is pre-duplicated: rows 0-63 and 64-127 identical, [c, (ky kx d)]
    w = nc.dram_tensor("w", [2 * C, 9 * D], f16, kind="ExternalInput").ap()
    b = nc.dram_tensor("b", [D, RPC, OW], f16, kind="ExternalInput").ap()
    y = nc.dram_tensor("y", [D, RPC, OW], f16, kind="ExternalOutput").ap()

    b_flat = b.rearrange("d r x -> d (r x)")
    y_flat = y.rearrange("d r x -> d (r x)")

    with tile.TileContext(nc) as tc:
        with ExitStack() as ctx:
            xp = ctx.enter_context(tc.tile_pool(name="xin", bufs=1))
            wp = ctx.enter_context(tc.tile_pool(name="wt", bufs=1))
            bp = ctx.enter_context(tc.tile_pool(name="bias", bufs=8))
            op = ctx.enter_context(tc.tile_pool(name="out", bufs=4))
            pp = ctx.enter_context(tc.tile_pool(name="ps", bufs=4, space="PSUM"))

            # HAM pre-warm: the PE clock sits at 1.2 GHz until ~3.4us of
            # sustained matmul activity.  Burn that window on dummy
            # matmuls over a memset scratch tile while the input DMA is
            # in flight, so the real stream runs at 2.4 GHz from pair 0.
            # The dummies alternate between the two PE row-halves so each
            # LDWEIGHTS pulls ahead of the in-flight matmul (~110ns per
            # dummy); the PSUM target shares the pa pool's first buffer.
            sc = wp.tile([128, 128], f16, tag="scratch")
            nc.gpsimd.memset(sc[:], 0.0)
            wps_a = pp.tile([128, OW], f32, tag="pa")
            wps_b = pp.tile([128, OW], f32, tag="pb")
            for i in range(46):
                h = slice(0, 64) if i % 2 == 0 else slice(64, 128)
                wps = wps_a if i % 2 == 0 else wps_b
                nc.tensor.matmul(
                    wps[:, 0:128], sc[h, :], sc[h, 0:128],
                    start=True, stop=True,
                )

            # critical path to the first matmul: ky=0 weights land first
            # on sync while input row 0 lands on scalar, so the tap-0
            # matmuls start as soon as both clear
            wt = wp.tile([128, 9 * D], f16)
            nc.sync.dma_start(wt[:, 0:3 * D], w[:, 0:3 * D])
            nc.sync.dma_start(wt[:, 3 * D:], w[:, 3 * D:])

            xin = xp.tile([128, IBAND * W], f16)
            bounds = [0, 1, 3, 7, 13, 20, 27, IBAND]
            for ci in range(len(bounds) - 1):
                r0, r1 = bounds[ci], bounds[ci + 1]
                nc.scalar.dma_start(
                    xin[:, r0 * W:r1 * W], x[:, r0 * W:r1 * W]
                )

            # bias rides both rings, interleaved with the stores so each
            # ring's FIFO byte order matches production order: groups 0-1
            # fully up front on sync (scalar is busy with x), then group
            # g+2's pair issues just before group g's matmuls
            bias_tiles = {}

            def load_bias(g, eng_a, eng_b):
                ra, rb = g * GROWS, HALF + g * GROWS
                ba = bp.tile([128, GROWS * OW], f16, tag="ba")
                eng_a.dma_start(ba[:], b_flat[:, ra * OW:(ra + GROWS) * OW])
                bb = bp.tile([128, GROWS * OW], f16, tag="bb")
                eng_b.dma_start(bb[:], b_flat[:, rb * OW:(rb + GROWS) * OW])
                bias_tiles[g] = (ba, bb)

            load_bias(0, nc.sync, nc.sync)
            load_bias(1, nc.sync, nc.sync)

            for g in range(GROUPS):
                ra = g * GROWS                 # band rows ra..ra+3  (strip 0)
                rb = HALF + ra                 # band rows rb..rb+3  (strip 1)
                if g + 2 < GROUPS:
                    load_bias(g + 2, nc.sync, nc.scalar)
                ba, bb = bias_tiles.pop(g)
                ya = op.tile([128, GROWS * OW], f16, tag="ya")
                yb = op.tile([128, GROWS * OW], f16, tag="yb")

                tail = g >= GROUPS - 2
                for j in range(GROWS):
                    yl = ra + j                # strip-local output row
                    pa = pp.tile([128, OW], f32, tag="pa")
                    pb = pp.tile([128, OW], f32, tag="pb")
                    for t in range(9):
                        ky, kx = divmod(t, 3)
                        off = (yl + ky) * W + kx
                        nc.tensor.matmul(
                            pa[:],
                            wt[0:64, t * D:(t + 1) * D],
                            xin[0:64, off:off + OW],
                            start=(t == 0), stop=(t == 8),
                        )
                        nc.tensor.matmul(
                            pb[:],
                            wt[64:128, t * D:(t + 1) * D],
                            xin[64:128, off:off + OW],
                            start=(t == 0), stop=(t == 8),
                        )
                    sl = slice(j * OW, (j + 1) * OW)
                    if g == GROUPS - 1 and j == GROWS - 1:
                        # the very last row: column-split the bias-add and
                        # the store so the final HBM write starts one DVE
                        # half-add (not two full adds) after the last
                        # matmul, interleaved across both rings
                        HC = OW // 2
                        for c0, c1 in ((0, HC), (HC, OW)):
                            ch = slice(j * OW + c0, j * OW + c1)
                            nc.vector.tensor_add(
                                yb[:, ch], pb[:, c0:c1], bb[:, ch]
                            )
                            nc.sync.dma_start(
                                y_flat[:, (rb + j) * OW + c0:
                                       (rb + j) * OW + c1],
                                yb[:, ch],
                            )
                            nc.vector.tensor_add(
                                ya[:, ch], pa[:, c0:c1], ba[:, ch]
                            )
                            nc.scalar.dma_start(
                                y_flat[:, (ra + j) * OW + c0:
                                       (ra + j) * OW + c1],
                                ya[:, ch],
                            )
                    else:
                        nc.vector.tensor_add(ya[:, sl], pa[:], ba[:, sl])
                        nc.vector.tensor_add(yb[:, sl], pb[:], bb[:, sl])
                        if tail:
                            # last two groups: store each row as soon as
                            # its add lands, one strip per ring
                            nc.scalar.dma_start(
                                y_flat[:, (ra + j) * OW:(ra + j + 1) * OW],
                                ya[:, sl],
                            )
                            nc.sync.dma_start(
                                y_flat[:, (rb + j) * OW:(rb + j + 1) * OW],
                                yb[:, sl],
                            )

                if not tail:
                    nc.sync.dma_start(
                        y_flat[:, ra * OW:(ra + GROWS) * OW], ya[:]
                    )
                    nc.scalar.dma_start(
                        y_flat[:, rb * OW:(rb + GROWS) * OW], yb[:]
                    )

    nc.compile()
    return nc


def kernel(input, kernels, biases):
    global LAST_RESULTS
    if "nc" not in _CACHE:
        _CACHE["nc"] = _build_program()
    nc = _CACHE["nc"]

    xr = np.asarray(input, dtype=np.float16)                   # [C, H, W]
    w1 = np.ascontiguousarray(
        np.asarray(kernels, dtype=np.float32).transpose(1, 2, 3, 0)
    ).reshape(C, 9 * D).astype(np.float16)
    wr = np.concatenate([w1, w1], axis=0)                      # [128, 9*D]
    br = np.asarray(biases, dtype=np.float16)

    in_maps = []
    for s in STARTS:
        band = xr[:, s:s + BAND, :]
        xs = np.concatenate(
            [band[:, 0:IBAND, :], band[:, HALF:HALF + IBAND, :]], axis=0
        ).reshape(2 * C, IBAND * W)
        in_maps.append({
            "x": np.ascontiguousarray(xs),
            "w": wr,
            "b": np.ascontiguousarray(br[:, s:s + RPC, :]),
        })

    res = bass_utils.run_bass_kernel_spmd(
        nc, in_maps, core_ids=list(range(NCORES))
    )
    LAST_RESULTS = res

    out = np.empty((D, OH, OW), np.float32)
    for i, s in enumerate(STARTS):
        out[:, s:s + RPC, :] = res.results[i]["y"].astype(np.float32)
    return out
